# revision 26
# baseline (speedup 1.0000x reference)
"""MultiHeadAttention TRN2 kernel: tensor-parallel over heads across 8 NeuronCores.

Problem (hardcoded): BS=2, QLEN=2048, DIM=1024, NHEADS=16, HEAD=64.
  q = split_heads(x @ q_w.T + q_b) / sqrt(64)
  s = q @ k.T + mask ; w = softmax(s) ; ctx = w @ v
  out = merge_heads(ctx) @ o_w.T + o_b

Sharding: core c computes heads {2c, 2c+1} (rows 128c:128c+128 of q/k/v weights,
cols 128c:128c+128 of o_w).  Each core emits a full-shape bf16 partial of the
output projection; the host sums the 8 partials and adds o_b.

Design (v5, merged single-stream schedule; ~214us HW vs 477us naive / 227us
phase-split):
- Engine floors per core: PE ~180us of matmul streams, ACT ~143us of exp (exp
  exists ONLY on the scalar engine).  A phase-split kernel leaves the scalar
  idle during projections and then paces the PE at exp rate.  Here EVERYTHING
  runs in one software-pipelined stream: one scores pair per "kt unit" from
  t~15us to the end, with projection passes / PV / output projection woven
  between as PE filler so the PE (the binding engine) never waits on exp.
- Scores pairs use tile_position row-bands (64-deep per head) and execute
  CONCURRENTLY on the PE (~0.31us/pair vs 0.43 serial).  The denominators
  broadcast (bc) also runs as two concurrent [64,64] tiles on disjoint
  row/col bands.
- V^T is produced by PE identity-transposes ([128,128] blocks, batched per
  group into one packed psum tile): DMA-transposes run 1.25us each serially
  on a shared ring and cannot feed PV once attention starts early; they also
  block same-ring stores.
- Softmax denominators ride inside PV via ones-columns in the V2 stationary
  (sums land on psum rows 64 / 0); 1/s = reciprocal_approx_fast on DVE.
- PSUM (8 banks): scores 2x[128,1024] + PV accum 1x[128,1024] + shared
  2x[128,512] (projection passes, bc, outproj, V-transposes all rotate
  through the shared tag).  NOTE: splitting scores into 3x[128,512] halves
  trips a hardware PE utilization throttle (k=4/8 HAM windows, ~50% PE for
  100us+) AND corrupts late qtiles -- do not.
- GPSIMD is kept COMPLETELY IDLE: its software-DGE descriptor generation
  (big DMA pulls, stores, even large memsets) triggers the same hardware
  util-limit throttle.  GPSIMD also cannot touch PSUM at all.
- DMA: both HWDGE queues (sync + scalar).  sync: wq/xt0 interleaved first
  (first matmul ~9us incl ~8.6us engine-boot floor), then wk/wv/xt2/wo/
  xt4/xt6 + steady-state output stores.  scalar: small constants, xt1, xt3
  up front (before the exp stream occupies the queue), xt5/xt7 issued
  mid-stream between early exps, tail stores (scalar is idle there).
- Chain per qtile i (2 steps behind scores): evict ctx+sums -> bc broadcast
  -> reciprocal -> normalize muls (DVE) -> outproj (PE) -> bf16 store.
  ct is stored per-step so outproj has no deadline: tt2/tt3 of qtiles 0-3
  are DEFERRED into the exp-paced lean end (steps 6-7) where the PE would
  otherwise idle, pulling every earlier pair (and hence the exp stream's
  finish) earlier.
- Step 7 compacts PV(6) 3-per-unit, runs chain(6) in-loop, and starts PV(7)
  at 2-per-unit so only PV(7) kts 10-15 + chain(7) drain in the tail, with
  tail psum evictions split DVE/scalar.
"""

import sys

if "/opt/trn_rl_repo" not in sys.path:
    sys.path.insert(0, "/opt/trn_rl_repo")

import math
from contextlib import ExitStack

import ml_dtypes
import numpy as np

import concourse.bass as bass
import concourse.tile as tile
from concourse import bacc, mybir
from concourse.bass_utils import run_bass_kernel_spmd


# ---- problem constants ----
BS, QLEN, DIM, NHEADS = 2, 2048, 1024, 16
HEAD = DIM // NHEADS            # 64
NTOK = BS * QLEN                # 4096
NCORES = 8
HPC = NHEADS // NCORES          # 2 heads per core
LDIM = HPC * HEAD               # 128 local dims per core
NKCH = DIM // 128               # 8 contraction chunks for projections
NTT = NTOK // 512               # 8 token groups of 512
NKT = QLEN // 128               # 16 key tiles per batch
QTW = 512                       # query tile width for attention
NQT = QLEN // QTW               # 4 query tiles per batch
NSTEP = BS * NQT                # 8 qtiles total

DT = mybir.dt.bfloat16          # matmul compute dtype
NPDT = ml_dtypes.bfloat16
F32 = mybir.dt.float32
AF = mybir.ActivationFunctionType

_cache = {}


def build_program():
    nc = bacc.Bacc("TRN2", target_bir_lowering=False, debug=False,
                   num_devices=NCORES)

    # host-pretiled x^T, partition-major: per token-group g, partition p
    # holds the 8KB row (c, t) -> one descriptor per partition per group.
    xt = nc.dram_tensor("xt", [NTT, 128, NKCH, 512], DT,
                        kind="ExternalInput").ap()
    wq = nc.dram_tensor("wq", [128, NKCH, LDIM], DT,
                        kind="ExternalInput").ap()
    wk = nc.dram_tensor("wk", [128, NKCH, LDIM], DT,
                        kind="ExternalInput").ap()
    wv = nc.dram_tensor("wv", [128, NKCH, LDIM], DT,
                        kind="ExternalInput").ap()
    wo = nc.dram_tensor("wo", [LDIM, DIM], DT, kind="ExternalInput").ap()
    qb = nc.dram_tensor("qb", [LDIM, 1], F32, kind="ExternalInput").ap()
    kb = nc.dram_tensor("kb", [LDIM, 1], F32, kind="ExternalInput").ap()
    vb = nc.dram_tensor("vb", [LDIM, 1], F32, kind="ExternalInput").ap()
    maskd = nc.dram_tensor("maskd", [128, BS * NKT], F32,
                           kind="ExternalInput").ap()
    identd = nc.dram_tensor("identd", [128, 128], DT,
                            kind="ExternalInput").ap()
    out = nc.dram_tensor("out", [NTOK, DIM], DT, kind="ExternalOutput").ap()

    with tile.TileContext(nc) as tc, ExitStack() as ctx:
        singles = ctx.enter_context(tc.tile_pool(name="singles", bufs=1))
        vtpool = ctx.enter_context(tc.tile_pool(name="vt", bufs=3))
        evict = ctx.enter_context(tc.tile_pool(name="evict", bufs=4))
        # PSUM: scores 2x[128,1024] (4 banks) + pv 1x[128,1024] (2 banks)
        # + shared 2x[128,512] (2 banks) = 8 banks.
        scp = ctx.enter_context(tc.tile_pool(name="scp", bufs=2, space="PSUM"))
        pvp = ctx.enter_context(tc.tile_pool(name="pvp", bufs=1, space="PSUM"))
        shp = ctx.enter_context(tc.tile_pool(name="shp", bufs=2, space="PSUM"))

        # --- resident SBUF tensors ---
        wq_sb = singles.tile([128, NKCH, LDIM], DT, tag="wq")
        wk_sb = singles.tile([128, NKCH, LDIM], DT, tag="wk")
        wv_sb = singles.tile([128, NKCH, LDIM], DT, tag="wv")
        wo_sb = singles.tile([LDIM, DIM], DT, tag="wo")
        qb_sb = singles.tile([LDIM, 1], F32, tag="qb")
        kb_sb = singles.tile([LDIM, 1], F32, tag="kb")
        vb_sb = singles.tile([LDIM, 1], F32, tag="vb")
        mask_sb = singles.tile([128, BS * NKT], F32, tag="mask")
        ident_sb = singles.tile([128, 128], DT, tag="ident")
        # selector stationaries for the sums broadcast, shaped for two
        # CONCURRENT [64,64] PE tiles on disjoint row/col bands:
        #   sel[64:128,0,:]: row-64 extractor -> bc partitions 0:64 (h0)
        #   sel[0:64, 1,:]: row-0 extractor -> bc partitions 64:128 (h1)
        sel_sb = singles.tile([128, 2, 64], DT, tag="sel")
        xt_sb = singles.tile([128, NTT, NKCH, 512], DT, tag="xt")
        qt_sb = singles.tile([128, NTOK], DT, tag="qt")
        kt_sb = singles.tile([128, NTOK], DT, tag="kt")
        # V2 stationaries (full 128-col; odd widths mis-load on hw):
        #   h0: [V(64) | ones@64 | zeros]  -> ctx rows 0..63, sums row 64
        #   h1: [ones@0 | zeros | V@64..127] -> ctx rows 64..127, sums row 0
        v2_sb = singles.tile([128, BS, NKT, 2, 128], DT, tag="v2")
        st_sb = singles.tile([128, 2, NKT, 2 * QTW], DT, tag="st")
        # unnormalized ctx + sums evicted from psum
        ctu_sb = singles.tile([128, 2, 2 * QTW], DT, tag="ctu")
        rc32_sb = singles.tile([128, 2, QTW], F32, tag="rc32")
        ct_sb = singles.tile([128, NSTEP, QTW], DT, tag="ct")

        # --- DMA preamble.  sync carries g0's critical path (first chunks
        # split out so the first matmul starts ~2us in); scalar carries only
        # xt1 so the exp stream is never blocked; gpsimd SWDGE pulls the
        # late b1 groups. ---
        nc.scalar.dma_start(qb_sb[:], qb[:])
        nc.scalar.dma_start(kb_sb[:], kb[:])
        nc.scalar.dma_start(vb_sb[:], vb[:])
        nc.scalar.dma_start(mask_sb[:], maskd[:])
        nc.scalar.dma_start(ident_sb[:], identd[:])
        nc.scalar.dma_start(xt_sb[:, 0, 4:NKCH], xt[0, :, 4:NKCH])
        nc.scalar.dma_start(xt_sb[:, 1], xt[1])
        nc.scalar.dma_start(xt_sb[:, 3], xt[3])
        nc.sync.dma_start(wq_sb[:], wq[:])
        nc.sync.dma_start(xt_sb[:, 0, 0:2], xt[0, :, 0:2])
        nc.sync.dma_start(wk_sb[:], wk[:])
        nc.sync.dma_start(xt_sb[:, 0, 2:4], xt[0, :, 2:4])
        nc.sync.dma_start(wv_sb[:], wv[:])
        nc.sync.dma_start(xt_sb[:, 2], xt[2])
        nc.sync.dma_start(wo_sb[:], wo[:])
        nc.sync.dma_start(xt_sb[:, 4], xt[4])
        nc.sync.dma_start(xt_sb[:, 6], xt[6])

        # init constants: gpsimd (idle at start) handles the big memsets so
        # the DVE queue reaches the first projection evictions promptly.
        nc.vector.memset(sel_sb[:], 0.0)
        nc.vector.memset(sel_sb[64:65, 0, :], 1.0)
        nc.vector.memset(sel_sb[0:1, 1, :], 1.0)
        nc.vector.memset(v2_sb[:, 0], 0.0)
        nc.vector.memset(v2_sb[:, 1], 0.0)
        nc.vector.memset(v2_sb[:, :, :, 0, HEAD:HEAD + 1], 1.0)
        nc.vector.memset(v2_sb[:, :, :, 1, 0:1], 1.0)
        nc.vector.memset(ctu_sb[:], 0.0)


        # ---------- emission helpers ----------
        def emit_pair(i, kt):
            """scores for qtile i, key tile kt (both heads, concurrent on
            the PE via 64-row tile bands), then exp on the scalar engine."""
            b, qt = i // NQT, i % NQT
            ks = slice(QLEN * b + 128 * kt, QLEN * b + 128 * (kt + 1))
            qsub = slice(QLEN * b + QTW * qt, QLEN * b + QTW * (qt + 1))
            m_ap = mask_sb[:, b * NKT + kt:b * NKT + kt + 1]
            s_ps = scp.tile([128, 1024], F32, tag="sc", name="sps")
            for h in range(2):
                hs = slice(HEAD * h, HEAD * (h + 1))
                nc.tensor.matmul(s_ps[:, 512 * h:512 * (h + 1)],
                                 kt_sb[hs, ks], qt_sb[hs, qsub],
                                 start=True, stop=True,
                                 tile_position=(HEAD * h, 0))
            nc.scalar.activation(st_sb[:, i % 2, kt, :], s_ps[:],
                                 AF.Exp, bias=m_ap)

        def emit_pv(i, kt, ps):
            b = i // NQT
            st0, sp0 = (kt == 0), (kt == NKT - 1)
            for h in range(2):
                nc.tensor.matmul(
                    ps[:, 512 * h:512 * (h + 1)],
                    v2_sb[:, b, kt, h, :],
                    st_sb[:, i % 2, kt, 512 * h:512 * (h + 1)],
                    start=st0, stop=sp0, skip_group_check=True)

        def emit_evicts(i, ps):
            """evict unnormalized ctx+sums from the PV accumulator (frees
            the pvp bank for the next qtile within ~0.8us)."""
            sl = i % 2
            nc.vector.tensor_copy(ctu_sb[0:65, sl, 0:512], ps[0:65, 0:512])
            nc.vector.tensor_copy(ctu_sb[64:128, sl, 512:1024],
                                  ps[64:128, 512:1024])
            nc.vector.tensor_copy(ctu_sb[0:1, sl, 512:1024],
                                  ps[0:1, 512:1024])

        def emit_bc(i):
            sl = i % 2
            bc = shp.tile([128, 512], F32, tag="sh")
            nc.tensor.matmul(bc[0:64, :], sel_sb[64:128, 0, :],
                             ctu_sb[64:128, sl, 0:512], start=True, stop=True,
                             skip_group_check=True, tile_position=(64, 0))
            nc.tensor.matmul(bc[64:128, :], sel_sb[0:64, 1, :],
                             ctu_sb[0:64, sl, 512:1024], start=True, stop=True,
                             skip_group_check=True, tile_position=(0, 64))
            return bc

        def emit_recip(i, bc):
            nc.vector.reciprocal_approx_fast(rc32_sb[:, i % 2, :], bc[:])

        def emit_muls(i):
            sl = i % 2
            nc.vector.tensor_mul(ct_sb[0:64, i, :], ctu_sb[0:64, sl, 0:512],
                                 rc32_sb[0:64, sl, :])
            nc.vector.tensor_mul(ct_sb[64:128, i, :],
                                 ctu_sb[64:128, sl, 512:1024],
                                 rc32_sb[64:128, sl, :])

        def emit_outproj(i, tt, tail=False):
            """output projection for tokens [tok0, tok0+128); bf16 partial
            stored on alternating DMA queues.  In the tail the j0 eviction
            runs on the (then-idle) scalar engine."""
            b, qt = i // NQT, i % NQT
            tok0 = QLEN * b + QTW * qt + 128 * tt
            o_sb = evict.tile([128, 1024], DT, tag="osb")
            for j in range(2):
                o_ps = shp.tile([128, 512], F32, tag="sh")
                nc.tensor.matmul(o_ps[:],
                                 ct_sb[:, i, 128 * tt:128 * (tt + 1)],
                                 wo_sb[:, 512 * j:512 * (j + 1)],
                                 start=True, stop=True)
                if tail and j == 0:
                    nc.scalar.activation(o_sb[:, 0:512], o_ps[:], AF.Copy)
                else:
                    nc.vector.tensor_copy(o_sb[:, 512 * j:512 * (j + 1)],
                                          o_ps[:])
            if tail and tt % 2 == 0:
                nc.scalar.dma_start(out[tok0:tok0 + 128, :], o_sb[:])
            else:
                nc.sync.dma_start(out[tok0:tok0 + 128, :], o_sb[:])

        # --- projection passes.  Each pass = 8 matmuls into one shared
        # [128,512] psum tile; emitted in two 4-matmul halves so the psum
        # slot is held only ~2 filler slots. ---
        def proj_half(g, kind, half, ps):
            w_sb = {"q": wq_sb, "k": wk_sb, "v": wv_sb}[kind]
            for c in range(4 * half, 4 * half + 4):
                nc.tensor.matmul(ps[:], w_sb[:, c, :], xt_sb[:, g, c, :],
                                 start=(c == 0), stop=(c == NKCH - 1),
                                 skip_group_check=True)

        def proj_evict(g, kind, ps):
            gs = slice(512 * g, 512 * (g + 1))
            if kind == "q":
                nc.vector.tensor_scalar_add(qt_sb[:, gs], ps[:], qb_sb[:, 0:1])
            elif kind == "k":
                nc.vector.tensor_scalar_add(kt_sb[:, gs], ps[:], kb_sb[:, 0:1])
            else:
                vt_t = vtpool.tile([128, 512], DT, tag="vtt")
                nc.vector.tensor_scalar_add(vt_t[:], ps[:], vb_sb[:, 0:1])
                return vt_t
            return None

        vt_hold = {}     # g -> vt sbuf tile awaiting transposes

        def emit_transposes(g):
            """4 PE identity-transposes [128,128] -> v2 stationaries."""
            b = g // (NTT // 2)
            vt_t = vt_hold.pop(g)
            tp = shp.tile([128, 512], DT, tag="sh")
            for t in range(4):
                nc.tensor.matmul(tp[:, 128 * t:128 * (t + 1)],
                                 vt_t[:, 128 * t:128 * (t + 1)],
                                 ident_sb[:], is_transpose=True,
                                 skip_group_check=True)
            for t in range(4):
                kt_i = (g % (NTT // 2)) * 4 + t
                nc.vector.tensor_copy(v2_sb[:, b, kt_i, 0, 0:HEAD],
                                      tp[:, 128 * t:128 * t + HEAD])
                nc.vector.tensor_copy(v2_sb[:, b, kt_i, 1, HEAD:128],
                                      tp[:, 128 * t + HEAD:128 * (t + 1)])

        # filler driver: each item is ('p', g, kind, half) or ('t', g)
        pass_ps = {}     # (g, kind) -> psum tile across the two halves

        def run_filler(item):
            if item[0] == "p":
                _, g, kind, half = item
                if half == 0:
                    pass_ps[(g, kind)] = shp.tile([128, 512], F32, tag="sh", name="pps")
                ps = pass_ps[(g, kind)]
                proj_half(g, kind, half, ps)
                if half == 1:
                    vt_t = proj_evict(g, kind, ps)
                    if vt_t is not None:
                        vt_hold[g] = vt_t
                    del pass_ps[(g, kind)]
            elif item[0] == "t":
                emit_transposes(item[1])
            else:
                emit_outproj(item[1], item[2])

        def P(g, kind):
            return [("p", g, kind, 0), ("p", g, kind, 1)]

        # per-step filler schedules (deadline-ordered; see module docstring)
        FILL = {
            0: (P(1, "k") + P(0, "v") + [("t", 0)] + P(2, "k") + P(3, "k")
                + P(1, "v") + [("t", 1)] + P(1, "q")),
            1: (P(2, "v") + [("t", 2)] + P(3, "v") + [("t", 3)]
                + P(2, "q")),
            2: (P(3, "q") + P(4, "k") + P(5, "k")),
            3: (P(4, "q") + P(5, "q") + P(4, "v")),
            4: (P(6, "k") + [("t", 4)] + P(7, "k") + P(5, "v")),
            5: ([("t", 5)] + P(6, "v") + [("t", 6)] + P(7, "v")
                + [("t", 7)] + P(6, "q")),
            6: (P(7, "q")
                + [("o", 0, 2), ("o", 0, 3), ("o", 1, 2), ("o", 1, 3),
                   ("o", 2, 2), ("o", 2, 3), ("o", 3, 2), ("o", 3, 3)]),
            7: [],
        }

        # ---------- pre-phase: g0 Q and K passes ----------
        for kind in ("q", "k"):
            ps = shp.tile([128, 512], F32, tag="sh")
            proj_half(0, kind, 0, ps)
            proj_half(0, kind, 1, ps)
            proj_evict(0, kind, ps)

        # ---------- main pipelined stream ----------
        # step s (s = 0..7): 16 units; unit u emits:
        #   pair(s, u);  PV(s-2, {13,14}) at u0, PV(s-2, 15) + evicts(s-2)
        #   at u1;  PV(s-1, u-3) for u>=3;  chain(s-2): bc@u4, recip@u5,
        #   muls@u6, outproj@u8/10/12/14;  fillers woven (deadline-ordered).
        # step 7 special: PV(6) compacted 2-per-unit at u3..10 (its exps are
        # all done), evicts(6)@u11, then PV(7) 2-per-unit from u12; the
        # remaining PV(7) kts + chains drain in the tail.
        pv_ps = [None, None]

        for s in range(NSTEP):
            fill = list(FILL[s])
            nf = len(fill)
            taken = 0
            last = s == NSTEP - 1
            for u in range(16):
                emit_pair(s, u)
                if s == 0 and u == 6:
                    nc.scalar.dma_start(xt_sb[:, 5], xt[5])
                if s == 0 and u == 12:
                    nc.scalar.dma_start(xt_sb[:, 7], xt[7])
                if u == 0 and s >= 2:
                    emit_pv(s - 2, 13, pv_ps[(s - 2) % 2])
                    emit_pv(s - 2, 14, pv_ps[(s - 2) % 2])
                if u == 1 and s >= 2:
                    emit_pv(s - 2, 15, pv_ps[(s - 2) % 2])
                    emit_evicts(s - 2, pv_ps[(s - 2) % 2])
                if u == 3 and s >= 1:
                    pv_ps[(s - 1) % 2] = pvp.tile([128, 1024], F32, tag="pv", name="pvt")
                if u >= 3 and s >= 1:
                    if not last:
                        emit_pv(s - 1, u - 3, pv_ps[(s - 1) % 2])
                    elif u <= 6:
                        for kk in range(4 * (u - 3), 4 * (u - 2)):
                            emit_pv(s - 1, kk, pv_ps[(s - 1) % 2])
                if last:
                    # PV(6) compacted 4/unit at u3..6; chain(6) follows
                    # immediately; PV(7) 2/unit from u9 so only kts 14-15
                    # + chain(7) land beyond the ~175us PE duty-cycle cliff.
                    if u == 7:
                        emit_evicts(s - 1, pv_ps[(s - 1) % 2])
                    if u == 8:
                        bcs6 = emit_bc(s - 1)
                    if u == 9:
                        emit_recip(s - 1, bcs6)
                        pv_ps[s % 2] = pvp.tile([128, 1024], F32, tag="pv", name="pvt")
                    if u >= 9:
                        emit_pv(s, 2 * (u - 9), pv_ps[s % 2])
                        emit_pv(s, 2 * (u - 9) + 1, pv_ps[s % 2])
                    if u == 10:
                        emit_muls(s - 1)
                    if u == 11:
                        emit_outproj(s - 1, 0)
                    if u == 13:
                        emit_outproj(s - 1, 1)
                # chain ops for qtile s-2
                if s >= 2:
                    if u == 4:
                        bcs = emit_bc(s - 2)
                    if u == 5:
                        emit_recip(s - 2, bcs)
                    if u == 6:
                        emit_muls(s - 2)
                    if u in (8, 10) or (s >= 6 and u in (12, 14)):
                        emit_outproj(s - 2, (u - 8) // 2)
                # fillers: spread evenly across the 16 units; in the last
                # step hold them past the chain-6 DVE ops so their psum
                # evictions don't delay PV(7)'s pvp handover
                if last:
                    want = 0 if u < 10 else (nf * (u - 9) + 5) // 6
                else:
                    want = (nf * (u + 1) + 15) // 16
                while taken < want:
                    run_filler(fill[taken])
                    taken += 1

        # ---------- tail ----------
        emit_pv(7, 14, pv_ps[1])
        emit_outproj(6, 2, tail=True)
        emit_pv(7, 15, pv_ps[1])
        emit_outproj(6, 3, tail=True)
        emit_evicts(7, pv_ps[1])
        bcs7 = emit_bc(7)
        emit_recip(7, bcs7)
        emit_muls(7)
        for tt in range(4):
            emit_outproj(7, tt, tail=True)

    nc.compile()
    return nc


def shard_inputs(input, mask, q_w, q_b, k_w, k_b, v_w, v_b, o_w, o_b):
    x = np.asarray(input, np.float32)
    # xt[g, p, c, t] = x[512 g + t, 128 c + p]
    xt = np.ascontiguousarray(
        x.reshape(NTT, 512, NKCH, 128).transpose(0, 3, 2, 1)).astype(NPDT)
    m = np.asarray(mask, np.float32).reshape(BS, NKT, 128)
    maskd = np.ascontiguousarray(m.transpose(2, 0, 1).reshape(128, BS * NKT))
    scale = 1.0 / math.sqrt(HEAD)
    ident = np.eye(128, dtype=NPDT)

    def pmaj(w):  # [1024, 128] -> [128, 8, 128] partition-major
        return np.ascontiguousarray(
            w.reshape(NKCH, 128, LDIM).transpose(1, 0, 2)).astype(NPDT)

    in_maps = []
    for c in range(NCORES):
        L = slice(LDIM * c, LDIM * (c + 1))
        in_maps.append({
            "xt": xt,
            "wq": pmaj((q_w[L, :] * scale).T),
            "wk": pmaj(k_w[L, :].T),
            "wv": pmaj(v_w[L, :].T),
            "wo": np.ascontiguousarray(o_w[:, L].T).astype(NPDT),
            "qb": (q_b[L] * scale).astype(np.float32).reshape(LDIM, 1),
            "kb": k_b[L].astype(np.float32).reshape(LDIM, 1),
            "vb": v_b[L].astype(np.float32).reshape(LDIM, 1),
            "maskd": maskd,
            "identd": ident,
        })
    return in_maps


def run(in_maps, **kw):
    if "nc" not in _cache:
        _cache["nc"] = build_program()
    return run_bass_kernel_spmd(_cache["nc"], in_maps,
                                core_ids=list(range(NCORES)), **kw)


def kernel(input, mask, q_w, q_b, k_w, k_b, v_w, v_b, o_w, o_b,
           bs=BS, qlen=QLEN):
    assert int(bs) == BS and int(qlen) == QLEN
    in_maps = shard_inputs(np.asarray(input), np.asarray(mask),
                           np.asarray(q_w), np.asarray(q_b),
                           np.asarray(k_w), np.asarray(k_b),
                           np.asarray(v_w), np.asarray(v_b),
                           np.asarray(o_w), np.asarray(o_b))
    res = run(in_maps)
    acc = np.zeros((NTOK, DIM), np.float32)
    for r in res.results:
        acc += np.asarray(r["out"], dtype=np.float32)
    acc += np.asarray(o_b, np.float32)[None, :]
    return acc


# revision 28
# speedup vs baseline: 1.0277x; 1.0277x over previous
"""MultiHeadAttention TRN2 kernel: tensor-parallel over heads across 8 NeuronCores.

Problem (hardcoded): BS=2, QLEN=2048, DIM=1024, NHEADS=16, HEAD=64.
  q = split_heads(x @ q_w.T + q_b) / sqrt(64)
  s = q @ k.T + mask ; w = softmax(s) ; ctx = w @ v
  out = merge_heads(ctx) @ o_w.T + o_b

Sharding: core c computes heads {2c, 2c+1} (rows 128c:128c+128 of q/k/v weights,
cols 128c:128c+128 of o_w).  Each core emits a full-shape bf16 partial of the
output projection; the host sums the 8 partials and adds o_b.

Design (v5, merged single-stream schedule; ~214us HW vs 477us naive / 227us
phase-split):
- Engine floors per core: PE ~180us of matmul streams, ACT ~143us of exp (exp
  exists ONLY on the scalar engine).  A phase-split kernel leaves the scalar
  idle during projections and then paces the PE at exp rate.  Here EVERYTHING
  runs in one software-pipelined stream: one scores pair per "kt unit" from
  t~15us to the end, with projection passes / PV / output projection woven
  between as PE filler so the PE (the binding engine) never waits on exp.
- Scores pairs use tile_position row-bands (64-deep per head) and execute
  CONCURRENTLY on the PE (~0.31us/pair vs 0.43 serial).  The denominators
  broadcast (bc) also runs as two concurrent [64,64] tiles on disjoint
  row/col bands.
- V^T is produced by PE identity-transposes ([128,128] blocks, batched per
  group into one packed psum tile): DMA-transposes run 1.25us each serially
  on a shared ring and cannot feed PV once attention starts early; they also
  block same-ring stores.
- Softmax denominators ride inside PV via ones-columns in the V2 stationary
  (sums land on psum rows 64 / 0); 1/s = reciprocal_approx_fast on DVE.
- PSUM (8 banks): scores 2x[128,1024] + PV accum 1x[128,1024] + shared
  2x[128,512] (projection passes, bc, outproj, V-transposes all rotate
  through the shared tag).  NOTE: splitting scores into 3x[128,512] halves
  trips a hardware PE utilization throttle (k=4/8 HAM windows, ~50% PE for
  100us+) AND corrupts late qtiles -- do not.
- GPSIMD is kept COMPLETELY IDLE: its software-DGE descriptor generation
  (big DMA pulls, stores, even large memsets) triggers the same hardware
  util-limit throttle.  GPSIMD also cannot touch PSUM at all.
- DMA: both HWDGE queues (sync + scalar).  sync: wq/xt0 interleaved first
  (first matmul ~9us incl ~8.6us engine-boot floor), then wk/wv/xt2/wo/
  xt4/xt6 + steady-state output stores.  scalar: small constants, xt1, xt3
  up front (before the exp stream occupies the queue), xt5/xt7 issued
  mid-stream between early exps, tail stores (scalar is idle there).
- Chain per qtile i (2 steps behind scores): evict ctx+sums -> bc broadcast
  -> reciprocal -> normalize muls (DVE) -> outproj (PE) -> bf16 store.
  ct is stored per-step so outproj has no deadline: tt2/tt3 of qtiles 0-3
  are DEFERRED into the exp-paced lean end (steps 6-7) where the PE would
  otherwise idle, pulling every earlier pair (and hence the exp stream's
  finish) earlier.
- Step 7 compacts PV(6) 3-per-unit, runs chain(6) in-loop, and starts PV(7)
  at 2-per-unit so only PV(7) kts 10-15 + chain(7) drain in the tail, with
  tail psum evictions split DVE/scalar.
"""

import sys

if "/opt/trn_rl_repo" not in sys.path:
    sys.path.insert(0, "/opt/trn_rl_repo")

import math
from contextlib import ExitStack

import ml_dtypes
import numpy as np

import concourse.bass as bass
import concourse.tile as tile
from concourse import bacc, mybir
from concourse.bass_utils import run_bass_kernel_spmd


# ---- problem constants ----
BS, QLEN, DIM, NHEADS = 2, 2048, 1024, 16
HEAD = DIM // NHEADS            # 64
NTOK = BS * QLEN                # 4096
NCORES = 8
HPC = NHEADS // NCORES          # 2 heads per core
LDIM = HPC * HEAD               # 128 local dims per core
NKCH = DIM // 128               # 8 contraction chunks for projections
NTT = NTOK // 512               # 8 token groups of 512
NKT = QLEN // 128               # 16 key tiles per batch
QTW = 512                       # query tile width for attention
NQT = QLEN // QTW               # 4 query tiles per batch
NSTEP = BS * NQT                # 8 qtiles total

DT = mybir.dt.bfloat16          # matmul compute dtype
NPDT = ml_dtypes.bfloat16
F32 = mybir.dt.float32
AF = mybir.ActivationFunctionType

_cache = {}


def build_program():
    nc = bacc.Bacc("TRN2", target_bir_lowering=False, debug=False,
                   num_devices=NCORES)

    # host-pretiled x^T, partition-major: per token-group g, partition p
    # holds the 8KB row (c, t) -> one descriptor per partition per group.
    xt = nc.dram_tensor("xt", [NTT, 128, NKCH, 512], DT,
                        kind="ExternalInput").ap()
    wq = nc.dram_tensor("wq", [128, NKCH, LDIM], DT,
                        kind="ExternalInput").ap()
    wk = nc.dram_tensor("wk", [128, NKCH, LDIM], DT,
                        kind="ExternalInput").ap()
    wv = nc.dram_tensor("wv", [128, NKCH, LDIM], DT,
                        kind="ExternalInput").ap()
    wo = nc.dram_tensor("wo", [LDIM, DIM], DT, kind="ExternalInput").ap()
    qb = nc.dram_tensor("qb", [LDIM, 1], F32, kind="ExternalInput").ap()
    kb = nc.dram_tensor("kb", [LDIM, 1], F32, kind="ExternalInput").ap()
    vb = nc.dram_tensor("vb", [LDIM, 1], F32, kind="ExternalInput").ap()
    maskd = nc.dram_tensor("maskd", [128, BS * NKT], F32,
                           kind="ExternalInput").ap()
    identd = nc.dram_tensor("identd", [128, 128], DT,
                            kind="ExternalInput").ap()
    out = nc.dram_tensor("out", [NTOK, DIM], DT, kind="ExternalOutput").ap()

    with tile.TileContext(nc) as tc, ExitStack() as ctx:
        singles = ctx.enter_context(tc.tile_pool(name="singles", bufs=1))
        vtpool = ctx.enter_context(tc.tile_pool(name="vt", bufs=3))
        evict = ctx.enter_context(tc.tile_pool(name="evict", bufs=4))
        # PSUM: scores 2x[128,1024] (4 banks) + pv 1x[128,1024] (2 banks)
        # + shared 2x[128,512] (2 banks) = 8 banks.
        scp = ctx.enter_context(tc.tile_pool(name="scp", bufs=2, space="PSUM"))
        pvp = ctx.enter_context(tc.tile_pool(name="pvp", bufs=1, space="PSUM"))
        shp = ctx.enter_context(tc.tile_pool(name="shp", bufs=2, space="PSUM"))

        # --- resident SBUF tensors ---
        wq_sb = singles.tile([128, NKCH, LDIM], DT, tag="wq")
        wk_sb = singles.tile([128, NKCH, LDIM], DT, tag="wk")
        wv_sb = singles.tile([128, NKCH, LDIM], DT, tag="wv")
        wo_sb = singles.tile([LDIM, DIM], DT, tag="wo")
        qb_sb = singles.tile([LDIM, 1], F32, tag="qb")
        kb_sb = singles.tile([LDIM, 1], F32, tag="kb")
        vb_sb = singles.tile([LDIM, 1], F32, tag="vb")
        mask_sb = singles.tile([128, BS * NKT], F32, tag="mask")
        ident_sb = singles.tile([128, 128], DT, tag="ident")
        # selector stationaries for the sums broadcast, shaped for two
        # CONCURRENT [64,64] PE tiles on disjoint row/col bands:
        #   sel[64:128,0,:]: row-64 extractor -> bc partitions 0:64 (h0)
        #   sel[0:64, 1,:]: row-0 extractor -> bc partitions 64:128 (h1)
        sel_sb = singles.tile([128, 2, 64], DT, tag="sel")
        xt_sb = singles.tile([128, NTT, NKCH, 512], DT, tag="xt")
        qt_sb = singles.tile([128, NTOK], DT, tag="qt")
        kt_sb = singles.tile([128, NTOK], DT, tag="kt")
        # V2 stationaries (full 128-col; odd widths mis-load on hw):
        #   h0: [V(64) | ones@64 | zeros]  -> ctx rows 0..63, sums row 64
        #   h1: [ones@0 | zeros | V@64..127] -> ctx rows 64..127, sums row 0
        v2_sb = singles.tile([128, BS, NKT, 2, 128], DT, tag="v2")
        st_sb = singles.tile([128, 2, NKT, 2 * QTW], DT, tag="st")
        # unnormalized ctx + sums evicted from psum
        ctu_sb = singles.tile([128, 2, 2 * QTW], DT, tag="ctu")
        rc32_sb = singles.tile([128, 2, QTW], F32, tag="rc32")
        ct_sb = singles.tile([128, NSTEP, QTW], DT, tag="ct")

        # --- DMA preamble.  sync carries g0's critical path (first chunks
        # split out so the first matmul starts ~2us in); scalar carries only
        # xt1 so the exp stream is never blocked; gpsimd SWDGE pulls the
        # late b1 groups. ---
        nc.scalar.dma_start(qb_sb[:], qb[:])
        nc.scalar.dma_start(kb_sb[:], kb[:])
        nc.scalar.dma_start(vb_sb[:], vb[:])
        nc.scalar.dma_start(mask_sb[:], maskd[:])
        nc.scalar.dma_start(ident_sb[:], identd[:])
        nc.scalar.dma_start(xt_sb[:, 0, 4:NKCH], xt[0, :, 4:NKCH])
        nc.scalar.dma_start(xt_sb[:, 1], xt[1])
        nc.scalar.dma_start(xt_sb[:, 3], xt[3])
        nc.sync.dma_start(wq_sb[:], wq[:])
        nc.sync.dma_start(xt_sb[:, 0, 0:2], xt[0, :, 0:2])
        nc.sync.dma_start(wk_sb[:], wk[:])
        nc.sync.dma_start(xt_sb[:, 0, 2:4], xt[0, :, 2:4])
        nc.sync.dma_start(wv_sb[:], wv[:])
        nc.sync.dma_start(xt_sb[:, 2], xt[2])
        nc.sync.dma_start(wo_sb[:], wo[:])
        nc.sync.dma_start(xt_sb[:, 4], xt[4])
        nc.sync.dma_start(xt_sb[:, 6], xt[6])

        # init constants: gpsimd (idle at start) handles the big memsets so
        # the DVE queue reaches the first projection evictions promptly.
        nc.vector.memset(sel_sb[:], 0.0)
        nc.vector.memset(sel_sb[64:65, 0, :], 1.0)
        nc.vector.memset(sel_sb[0:1, 1, :], 1.0)
        nc.vector.memset(v2_sb[:, 0], 0.0)
        nc.vector.memset(v2_sb[:, 1], 0.0)
        nc.vector.memset(v2_sb[:, :, :, 0, HEAD:HEAD + 1], 1.0)
        nc.vector.memset(v2_sb[:, :, :, 1, 0:1], 1.0)
        nc.vector.memset(ctu_sb[:], 0.0)


        # ---------- emission helpers ----------
        def emit_pair(i, kt):
            """scores for qtile i, key tile kt (both heads, concurrent on
            the PE via 64-row tile bands), then exp on the scalar engine."""
            b, qt = i // NQT, i % NQT
            ks = slice(QLEN * b + 128 * kt, QLEN * b + 128 * (kt + 1))
            qsub = slice(QLEN * b + QTW * qt, QLEN * b + QTW * (qt + 1))
            m_ap = mask_sb[:, b * NKT + kt:b * NKT + kt + 1]
            s_ps = scp.tile([128, 1024], F32, tag="sc", name="sps")
            for h in range(2):
                hs = slice(HEAD * h, HEAD * (h + 1))
                nc.tensor.matmul(s_ps[:, 512 * h:512 * (h + 1)],
                                 kt_sb[hs, ks], qt_sb[hs, qsub],
                                 start=True, stop=True,
                                 tile_position=(HEAD * h, 0))
            nc.scalar.activation(st_sb[:, i % 2, kt, :], s_ps[:],
                                 AF.Exp, bias=m_ap)

        def emit_pv(i, kt, ps):
            b = i // NQT
            st0, sp0 = (kt == 0), (kt == NKT - 1)
            for h in range(2):
                nc.tensor.matmul(
                    ps[:, 512 * h:512 * (h + 1)],
                    v2_sb[:, b, kt, h, :],
                    st_sb[:, i % 2, kt, 512 * h:512 * (h + 1)],
                    start=st0, stop=sp0, skip_group_check=True)

        def emit_evicts(i, ps):
            """evict unnormalized ctx+sums from the PV accumulator (frees
            the pvp bank for the next qtile within ~0.8us)."""
            sl = i % 2
            nc.vector.tensor_copy(ctu_sb[0:65, sl, 0:512], ps[0:65, 0:512])
            nc.vector.tensor_copy(ctu_sb[64:128, sl, 512:1024],
                                  ps[64:128, 512:1024])
            nc.vector.tensor_copy(ctu_sb[0:1, sl, 512:1024],
                                  ps[0:1, 512:1024])

        def emit_bc(i):
            sl = i % 2
            bc = shp.tile([128, 512], F32, tag="sh")
            nc.tensor.matmul(bc[0:64, :], sel_sb[64:128, 0, :],
                             ctu_sb[64:128, sl, 0:512], start=True, stop=True,
                             skip_group_check=True, tile_position=(64, 0))
            nc.tensor.matmul(bc[64:128, :], sel_sb[0:64, 1, :],
                             ctu_sb[0:64, sl, 512:1024], start=True, stop=True,
                             skip_group_check=True, tile_position=(0, 64))
            return bc

        def emit_recip(i, bc):
            nc.vector.reciprocal_approx_fast(rc32_sb[:, i % 2, :], bc[:])

        def emit_muls(i):
            sl = i % 2
            nc.vector.tensor_mul(ct_sb[0:64, i, :], ctu_sb[0:64, sl, 0:512],
                                 rc32_sb[0:64, sl, :])
            nc.vector.tensor_mul(ct_sb[64:128, i, :],
                                 ctu_sb[64:128, sl, 512:1024],
                                 rc32_sb[64:128, sl, :])

        def emit_outproj(i, tt, tail=False):
            """output projection for tokens [tok0, tok0+128); bf16 partial
            stored on alternating DMA queues.  In the tail the j0 eviction
            runs on the (then-idle) scalar engine."""
            b, qt = i // NQT, i % NQT
            tok0 = QLEN * b + QTW * qt + 128 * tt
            o_sb = evict.tile([128, 1024], DT, tag="osb")
            for j in range(2):
                o_ps = shp.tile([128, 512], F32, tag="sh")
                nc.tensor.matmul(o_ps[:],
                                 ct_sb[:, i, 128 * tt:128 * (tt + 1)],
                                 wo_sb[:, 512 * j:512 * (j + 1)],
                                 start=True, stop=True)
                if tail and j == 0:
                    nc.scalar.activation(o_sb[:, 0:512], o_ps[:], AF.Copy)
                else:
                    nc.vector.tensor_copy(o_sb[:, 512 * j:512 * (j + 1)],
                                          o_ps[:])
            if tail and tt % 2 == 0:
                nc.scalar.dma_start(out[tok0:tok0 + 128, :], o_sb[:])
            else:
                nc.sync.dma_start(out[tok0:tok0 + 128, :], o_sb[:])

        # --- projection passes.  Each pass = 8 matmuls into one shared
        # [128,512] psum tile; emitted in two 4-matmul halves so the psum
        # slot is held only ~2 filler slots. ---
        def proj_half(g, kind, half, ps):
            w_sb = {"q": wq_sb, "k": wk_sb, "v": wv_sb}[kind]
            for c in range(4 * half, 4 * half + 4):
                nc.tensor.matmul(ps[:], w_sb[:, c, :], xt_sb[:, g, c, :],
                                 start=(c == 0), stop=(c == NKCH - 1),
                                 skip_group_check=True)

        def proj_evict(g, kind, ps):
            gs = slice(512 * g, 512 * (g + 1))
            if kind == "q":
                nc.vector.tensor_scalar_add(qt_sb[:, gs], ps[:], qb_sb[:, 0:1])
            elif kind == "k":
                nc.vector.tensor_scalar_add(kt_sb[:, gs], ps[:], kb_sb[:, 0:1])
            else:
                vt_t = vtpool.tile([128, 512], DT, tag="vtt")
                nc.vector.tensor_scalar_add(vt_t[:], ps[:], vb_sb[:, 0:1])
                return vt_t
            return None

        vt_hold = {}     # g -> vt sbuf tile awaiting transposes

        def emit_transposes(g):
            """4 PE identity-transposes [128,128] -> v2 stationaries."""
            b = g // (NTT // 2)
            vt_t = vt_hold.pop(g)
            tp = shp.tile([128, 512], DT, tag="sh")
            for t in range(4):
                nc.tensor.matmul(tp[:, 128 * t:128 * (t + 1)],
                                 vt_t[:, 128 * t:128 * (t + 1)],
                                 ident_sb[:], is_transpose=True,
                                 skip_group_check=True)
            for t in range(4):
                kt_i = (g % (NTT // 2)) * 4 + t
                nc.vector.tensor_copy(v2_sb[:, b, kt_i, 0, 0:HEAD],
                                      tp[:, 128 * t:128 * t + HEAD])
                nc.vector.tensor_copy(v2_sb[:, b, kt_i, 1, HEAD:128],
                                      tp[:, 128 * t + HEAD:128 * (t + 1)])

        # filler driver: each item is ('p', g, kind, half) or ('t', g)
        pass_ps = {}     # (g, kind) -> psum tile across the two halves

        def run_filler(item):
            if item[0] == "p":
                _, g, kind, half = item
                if half == 0:
                    pass_ps[(g, kind)] = shp.tile([128, 512], F32, tag="sh", name="pps")
                ps = pass_ps[(g, kind)]
                proj_half(g, kind, half, ps)
                if half == 1:
                    vt_t = proj_evict(g, kind, ps)
                    if vt_t is not None:
                        vt_hold[g] = vt_t
                    del pass_ps[(g, kind)]
            elif item[0] == "t":
                emit_transposes(item[1])
            else:
                emit_outproj(item[1], item[2])

        def P(g, kind):
            return [("p", g, kind, 0), ("p", g, kind, 1)]

        # per-step filler schedules (deadline-ordered; see module docstring)
        FILL = {
            0: (P(1, "k") + P(0, "v") + [("t", 0)] + P(2, "k") + P(3, "k")
                + P(1, "v") + [("t", 1)] + P(1, "q")),
            1: (P(2, "v") + [("t", 2)] + P(3, "v") + [("t", 3)]
                + P(2, "q")),
            2: (P(3, "q") + P(4, "k") + P(5, "k")),
            3: (P(4, "q") + P(5, "q") + P(4, "v")),
            4: (P(6, "k") + [("t", 4)] + P(7, "k") + P(5, "v")),
            5: ([("t", 5)] + P(6, "v") + [("t", 6)] + P(7, "v")
                + [("t", 7)] + P(6, "q")),
            6: (P(7, "q")
                + [("o", 0, 2), ("o", 0, 3), ("o", 1, 2)]),
            7: [("o", 1, 3), ("o", 2, 2)],
        }

        # ---------- pre-phase: g0 Q and K passes ----------
        for kind in ("q", "k"):
            ps = shp.tile([128, 512], F32, tag="sh")
            proj_half(0, kind, 0, ps)
            proj_half(0, kind, 1, ps)
            proj_evict(0, kind, ps)

        # ---------- main pipelined stream ----------
        # step s (s = 0..7): 16 units; unit u emits:
        #   pair(s, u);  PV(s-2, {13,14}) at u0, PV(s-2, 15) + evicts(s-2)
        #   at u1;  PV(s-1, u-3) for u>=3;  chain(s-2): bc@u4, recip@u5,
        #   muls@u6, outproj@u8/10/12/14;  fillers woven (deadline-ordered).
        # step 7 special: PV(6) compacted 2-per-unit at u3..10 (its exps are
        # all done), evicts(6)@u11, then PV(7) 2-per-unit from u12; the
        # remaining PV(7) kts + chains drain in the tail.
        pv_ps = [None, None]

        for s in range(NSTEP):
            fill = list(FILL[s])
            nf = len(fill)
            taken = 0
            last = s == NSTEP - 1
            for u in range(16):
                emit_pair(s, u)
                if s == 0 and u == 6:
                    nc.scalar.dma_start(xt_sb[:, 5], xt[5])
                if s == 0 and u == 12:
                    nc.scalar.dma_start(xt_sb[:, 7], xt[7])
                if u == 0 and s >= 2:
                    emit_pv(s - 2, 13, pv_ps[(s - 2) % 2])
                    emit_pv(s - 2, 14, pv_ps[(s - 2) % 2])
                if u == 1 and s >= 2:
                    emit_pv(s - 2, 15, pv_ps[(s - 2) % 2])
                    emit_evicts(s - 2, pv_ps[(s - 2) % 2])
                if u == 3 and s >= 1:
                    pv_ps[(s - 1) % 2] = pvp.tile([128, 1024], F32, tag="pv", name="pvt")
                if u >= 3 and s >= 1:
                    if not last:
                        emit_pv(s - 1, u - 3, pv_ps[(s - 1) % 2])
                    elif u <= 6:
                        for kk in range(4 * (u - 3), 4 * (u - 2)):
                            emit_pv(s - 1, kk, pv_ps[(s - 1) % 2])
                if last:
                    # PV(6) compacted 4/unit at u3..6; chain(6) follows
                    # immediately; PV(7) 2/unit from u9 so only kts 14-15
                    # + chain(7) land beyond the ~175us PE duty-cycle cliff.
                    if u == 7:
                        emit_evicts(s - 1, pv_ps[(s - 1) % 2])
                    if u == 8:
                        bcs6 = emit_bc(s - 1)
                    if u == 9:
                        emit_recip(s - 1, bcs6)
                        pv_ps[s % 2] = pvp.tile([128, 1024], F32, tag="pv", name="pvt")
                    if u >= 9:
                        emit_pv(s, 2 * (u - 9), pv_ps[s % 2])
                        emit_pv(s, 2 * (u - 9) + 1, pv_ps[s % 2])
                    if u == 10:
                        emit_muls(s - 1)
                    if u == 11:
                        emit_outproj(s - 1, 0)
                    if u == 13:
                        emit_outproj(s - 1, 1)
                # chain ops for qtile s-2
                if s >= 2:
                    if u == 4:
                        bcs = emit_bc(s - 2)
                    if u == 5:
                        emit_recip(s - 2, bcs)
                    if u == 6:
                        emit_muls(s - 2)
                    if u in (8, 10) or (s >= 6 and u in (12, 14)):
                        emit_outproj(s - 2, (u - 8) // 2)
                # fillers: spread evenly across the 16 units; in the last
                # step hold them past the chain-6 DVE ops so their psum
                # evictions don't delay PV(7)'s pvp handover
                if last:
                    want = 0 if u < 10 else (nf * (u - 9) + 5) // 6
                else:
                    want = (nf * (u + 1) + 15) // 16
                while taken < want:
                    run_filler(fill[taken])
                    taken += 1

        # ---------- tail ----------
        emit_pv(7, 14, pv_ps[1])
        emit_outproj(6, 2, tail=True)
        emit_pv(7, 15, pv_ps[1])
        emit_outproj(2, 3)
        emit_outproj(6, 3, tail=True)
        emit_evicts(7, pv_ps[1])
        bcs7 = emit_bc(7)
        emit_recip(7, bcs7)
        emit_muls(7)
        emit_outproj(3, 2)
        emit_outproj(3, 3)
        for tt in range(4):
            emit_outproj(7, tt, tail=True)

    nc.compile()
    return nc


def shard_inputs(input, mask, q_w, q_b, k_w, k_b, v_w, v_b, o_w, o_b):
    x = np.asarray(input, np.float32)
    # xt[g, p, c, t] = x[512 g + t, 128 c + p]
    xt = np.ascontiguousarray(
        x.reshape(NTT, 512, NKCH, 128).transpose(0, 3, 2, 1)).astype(NPDT)
    m = np.asarray(mask, np.float32).reshape(BS, NKT, 128)
    maskd = np.ascontiguousarray(m.transpose(2, 0, 1).reshape(128, BS * NKT))
    scale = 1.0 / math.sqrt(HEAD)
    ident = np.eye(128, dtype=NPDT)

    def pmaj(w):  # [1024, 128] -> [128, 8, 128] partition-major
        return np.ascontiguousarray(
            w.reshape(NKCH, 128, LDIM).transpose(1, 0, 2)).astype(NPDT)

    in_maps = []
    for c in range(NCORES):
        L = slice(LDIM * c, LDIM * (c + 1))
        in_maps.append({
            "xt": xt,
            "wq": pmaj((q_w[L, :] * scale).T),
            "wk": pmaj(k_w[L, :].T),
            "wv": pmaj(v_w[L, :].T),
            "wo": np.ascontiguousarray(o_w[:, L].T).astype(NPDT),
            "qb": (q_b[L] * scale).astype(np.float32).reshape(LDIM, 1),
            "kb": k_b[L].astype(np.float32).reshape(LDIM, 1),
            "vb": v_b[L].astype(np.float32).reshape(LDIM, 1),
            "maskd": maskd,
            "identd": ident,
        })
    return in_maps


def run(in_maps, **kw):
    if "nc" not in _cache:
        _cache["nc"] = build_program()
    return run_bass_kernel_spmd(_cache["nc"], in_maps,
                                core_ids=list(range(NCORES)), **kw)


def kernel(input, mask, q_w, q_b, k_w, k_b, v_w, v_b, o_w, o_b,
           bs=BS, qlen=QLEN):
    assert int(bs) == BS and int(qlen) == QLEN
    in_maps = shard_inputs(np.asarray(input), np.asarray(mask),
                           np.asarray(q_w), np.asarray(q_b),
                           np.asarray(k_w), np.asarray(k_b),
                           np.asarray(v_w), np.asarray(v_b),
                           np.asarray(o_w), np.asarray(o_b))
    res = run(in_maps)
    acc = np.zeros((NTOK, DIM), np.float32)
    for r in res.results:
        acc += np.asarray(r["out"], dtype=np.float32)
    acc += np.asarray(o_b, np.float32)[None, :]
    return acc
